# revision 2
# baseline (speedup 1.0000x reference)
"""GCLSTM (Chebyshev K=3 graph-conv LSTM gates) forward on 8 Trainium2 NeuronCores.

Math (derived from the reference model): the scan carry is unused and H/C start
at zero inside each step, so the output depends only on the LAST timestep and
every _cheb(H, ...) term reduces to its bias. What remains per output row i:

    deg[i]  = sum_{e: row[e]=i} w[e]
    dis     = deg > 0 ? 1/sqrt(max(deg, 1e-30)) : 0
    Y       = dis * X                      (row scaling)
    U1      = S(Y)       where  S(Z)[i] = sum_{e: row[e]=i} w[e] * Z[col[e]]
    Tx1     = -dis * U1
    U2      = S(dis^2 * U1)
    Tx2     = 2 * dis * U2 - X
    G_g     = X@(W[g,0]-W[g,2]) + Tx1@W[g,1] + (2*dis*U2)@W[g,2] + bias_g
    I = sigmoid(G_i); Tt = tanh(G_c); C = I*Tt
    O = sigmoid(G_o + wc[2]*C);  out = relu(O * tanh(C))

Sharding: nodes are 1-D partitioned across the 8 cores (rows of the
segment-sum stay local); gathered node features are exchanged via an
on-chip AllGather; the small 128x128 gate weights are replicated.

The per-edge scatter-add is performed as a dense matmul against a one-hot
"staircase" matrix built on the vector engine from the edge row indices, with
edges (pre-bucketed on the host by (row-block, col-half)) as the contraction
dimension; the per-edge gather of node features uses the SWDGE dma_gather
custom instruction (int16 indices, hence the col-half split).
"""

import numpy as np

P = 128
NCORES = 8

# ----------------------------------------------------------------------------
# Host-side sharding / bucketing
# ----------------------------------------------------------------------------


def _preprocess(X, row, col, w):
    """Bucket edges by (owner core, row block, col half); build device inputs."""
    N, F = X.shape
    assert F == P
    R = -(-N // NCORES)              # rows owned per core
    RB = -(-R // P)                  # 128-row blocks per core
    R_PAD = RB * P
    NFULL = NCORES * R_PAD           # rows of the (padded) allgathered table
    HALF = NFULL // 2
    assert HALF <= 32768, "int16 gather index limit"

    core = (row // R).astype(np.int64)
    lrow = (row - core * R).astype(np.int64)          # 0..R-1
    colc = col // R
    col_p = (colc * R_PAD + (col - colc * R)).astype(np.int64)  # padded global id

    blk = lrow // P                                   # row block 0..RB-1
    half = (col_p >= HALF).astype(np.int64)
    key = blk * 2 + half

    # group counts per (core, block, half)
    cnt = np.zeros((NCORES, RB, 2), np.int64)
    np.add.at(cnt, (core, blk, half), 1)
    G = np.maximum(1, -(-cnt.max(axis=0) // P))       # [RB, 2] groups, >=1
    Lseg = G * P                                      # padded edges per segment
    seg_start = np.concatenate([[0], np.cumsum(Lseg.ravel())])[:-1].reshape(RB, 2)
    TOT = int(Lseg.sum())                             # padded edges per core
    TG = TOT // P                                     # total groups per core

    deg_pad = 4
    in_maps = []
    for c in range(NCORES):
        sel = core == c
        lr_c = lrow[sel]
        cp_c = col_p[sel]
        w_c = w[sel]
        k_c = key[sel]

        order = np.argsort(k_c, kind="stable")
        lr_s, cp_s, w_s, k_s = lr_c[order], cp_c[order], w_c[order], k_c[order]
        cseg = np.bincount(k_s, minlength=RB * 2)
        within = np.arange(len(k_s)) - np.repeat(
            np.concatenate([[0], np.cumsum(cseg)])[:-1], cseg
        )
        pos = seg_start.ravel()[k_s] + within

        colp_arr = np.zeros(TOT, np.int64)
        w_arr = np.zeros(TOT, np.float32)
        lr_arr = np.zeros(TOT, np.float32)
        colp_arr[pos] = cp_s - (cp_s >= HALF) * HALF
        w_arr[pos] = w_s
        lr_arr[pos] = (lr_s - (lr_s // P) * P).astype(np.float32)

        idx16 = colp_arr.reshape(-1, 16).T            # [16, TOT/16]
        idx_all = np.tile(idx16, (8, 1)).astype(np.int16)
        lr_all = np.ascontiguousarray(lr_arr.reshape(-1, P).T)   # [128, TG]
        w_all = np.ascontiguousarray(w_arr.reshape(-1, P).T)

        # per-row padded weight lists for the degree reduction
        dmax = int(np.bincount(lr_c, minlength=R).max()) if len(lr_c) else 0
        deg_pad = max(deg_pad, -(-max(dmax, 1) // 4) * 4)

        in_maps.append(
            dict(idx_all=idx_all, lr_all=lr_all, w_all=w_all,
                 _lr_c=lr_c, _w_c=w_c)
        )

    for c in range(NCORES):
        m = in_maps[c]
        lr_c, w_c = m.pop("_lr_c"), m.pop("_w_c")
        order = np.argsort(lr_c, kind="stable")
        lr_s, w_s = lr_c[order], w_c[order]
        crow = np.bincount(lr_s, minlength=R_PAD)
        starts = np.concatenate([[0], np.cumsum(crow)])[:-1]
        rank = np.arange(len(lr_s)) - np.repeat(starts, crow)
        wdm = np.zeros((R_PAD, deg_pad), np.float32)
        wdm[lr_s, rank] = w_s
        m["w_deg"] = np.ascontiguousarray(
            wdm.reshape(RB, P, deg_pad).transpose(1, 0, 2).reshape(P, RB * deg_pad)
        )
        xl = np.zeros((R_PAD, P), np.float32)
        lo, hi = c * R, min((c + 1) * R, N)
        xl[: hi - lo] = X[lo:hi]
        m["x_loc"] = xl

    cfg = dict(N=N, R=R, RB=RB, R_PAD=R_PAD, NFULL=NFULL, HALF=HALF,
               DEG_PAD=deg_pad, TG=TG,
               G=G, seg_start=seg_start)
    return in_maps, cfg


# ----------------------------------------------------------------------------
# Device kernel
# ----------------------------------------------------------------------------


def _build(cfg):
    import concourse.bacc as bacc
    import concourse.mybir as mybir
    import concourse.tile as tile
    from concourse.masks import make_identity

    RB, DEG_PAD, TG = cfg["RB"], cfg["DEG_PAD"], cfg["TG"]
    R_PAD, NFULL, HALF = cfg["R_PAD"], cfg["NFULL"], cfg["HALF"]
    G = cfg["G"]
    seg_start = cfg["seg_start"]
    f32 = mybir.dt.float32
    Alu = mybir.AluOpType
    Act = mybir.ActivationFunctionType
    GATES = (0, 2, 3)  # i, c, o

    nc = bacc.Bacc("TRN2", target_bir_lowering=False, debug=False,
                   num_devices=NCORES)

    x_loc = nc.dram_tensor("x_loc", [R_PAD, P], f32, kind="ExternalInput")
    w_deg = nc.dram_tensor("w_deg", [P, RB * DEG_PAD], f32, kind="ExternalInput")
    idx_all = nc.dram_tensor("idx_all", [P, TG * 8], mybir.dt.int16, kind="ExternalInput")
    lr_all = nc.dram_tensor("lr_all", [P, TG], f32, kind="ExternalInput")
    w_all = nc.dram_tensor("w_all", [P, TG], f32, kind="ExternalInput")
    wx_t = nc.dram_tensor("wx_t", [4, 3, P, P], f32, kind="ExternalInput")
    bsum_t = nc.dram_tensor("bsum_t", [1, 4 * P], f32, kind="ExternalInput")
    wc_t = nc.dram_tensor("wc_t", [1, 3 * P], f32, kind="ExternalInput")
    out_loc = nc.dram_tensor("out_loc", [R_PAD, P], f32, kind="ExternalOutput")

    x_r = x_loc.rearrange("(b p) f -> p b f", p=P)
    out_r = out_loc.rearrange("(b p) f -> p b f", p=P)

    with tile.TileContext(nc) as tc:
        with (
            tc.tile_pool(name="const", bufs=1) as const,
            tc.tile_pool(name="pers", bufs=1) as pers,
            tc.tile_pool(name="work", bufs=3) as work,
            tc.tile_pool(name="vpool", bufs=3) as vpool,
            tc.tile_pool(name="mtpool", bufs=3) as mtpool,
            tc.tile_pool(name="ppool", bufs=2, space="PSUM") as ppool,
            tc.tile_pool(name="tpsum", bufs=2, space="PSUM") as tpsum,
            tc.tile_pool(name="gpsum", bufs=3, space="PSUM") as gpsum,
            tc.tile_pool(name="dram", bufs=1, space="DRAM") as dram,
        ):
            # ---------------- constants ----------------
            ident = const.tile([P, P], f32)
            make_identity(nc, ident[:])
            iota_i = const.tile([P, P], mybir.dt.int32)
            nc.gpsimd.iota(iota_i[:], pattern=[[1, P]], base=0, channel_multiplier=0)
            iota_f = const.tile([P, P], f32)
            nc.vector.tensor_copy(out=iota_f[:], in_=iota_i[:])
            ones1 = const.tile([1, P], f32)
            nc.vector.memset(ones1[:], 1.0)

            # gate weights (replicated, small)
            wsb = {}
            for g in GATES:
                for k in (1, 2):
                    t = const.tile([P, P], f32, tag=f"w{g}{k}")
                    nc.sync.dma_start(out=t[:], in_=wx_t[g, k])
                    wsb[(g, k)] = t
            w0m = {}
            for g in GATES:
                t0 = work.tile([P, P], f32, tag="wtmp")
                nc.sync.dma_start(out=t0[:], in_=wx_t[g, 0])
                t = const.tile([P, P], f32, tag=f"w0m{g}")
                nc.vector.tensor_tensor(out=t[:], in0=t0[:], in1=wsb[(g, 2)][:],
                                        op=Alu.subtract)
                w0m[g] = t

            # summed gate biases (bx+bh+bg precombined on host into bsum_t)
            bias_sb = const.tile([1, 4 * P], f32)
            nc.sync.dma_start(out=bias_sb[:], in_=bsum_t[:])
            wc_sb = const.tile([1, 3 * P], f32)
            nc.sync.dma_start(out=wc_sb[:], in_=wc_t[:])
            # replicate wc[2] across partitions with a K=1 matmul
            wc2_ps = tpsum.tile([P, P], f32, tag="tp")
            nc.tensor.matmul(out=wc2_ps[:], lhsT=ones1[:],
                             rhs=wc_sb[:, 2 * P:3 * P], start=True, stop=True)
            wc2_rep = const.tile([P, P], f32)
            nc.scalar.copy(out=wc2_rep[:], in_=wc2_ps[:])

            # ---------------- degree / dis ----------------
            wdeg_sb = pers.tile([P, RB * DEG_PAD], f32, tag="wdeg")
            nc.sync.dma_start(out=wdeg_sb[:], in_=w_deg[:])
            deg = const.tile([P, RB], f32)
            nc.vector.tensor_reduce(
                out=deg[:], in_=wdeg_sb[:].rearrange("p (b d) -> p b d", d=DEG_PAD),
                axis=mybir.AxisListType.X, op=Alu.add)
            dmax = const.tile([P, RB], f32)
            nc.vector.tensor_scalar(out=dmax[:], in0=deg[:], scalar1=1e-30,
                                    scalar2=None, op0=Alu.max)
            dsq = const.tile([P, RB], f32)
            nc.scalar.sqrt(out=dsq[:], in_=dmax[:])
            drec = const.tile([P, RB], f32)
            nc.vector.reciprocal(out=drec[:], in_=dsq[:])
            dpos = const.tile([P, RB], f32)
            nc.vector.tensor_scalar(out=dpos[:], in0=deg[:], scalar1=0.0,
                                    scalar2=None, op0=Alu.is_gt)
            dis = const.tile([P, RB], f32)
            nc.vector.tensor_tensor(out=dis[:], in0=drec[:], in1=dpos[:], op=Alu.mult)
            dis2 = const.tile([P, RB], f32)
            nc.vector.tensor_tensor(out=dis2[:], in0=dis[:], in1=dis[:], op=Alu.mult)
            ndis = const.tile([P, RB], f32)
            nc.vector.tensor_scalar(out=ndis[:], in0=dis[:], scalar1=-1.0,
                                    scalar2=None, op0=Alu.mult)
            dis2x = const.tile([P, RB], f32)
            nc.vector.tensor_scalar(out=dis2x[:], in0=dis[:], scalar1=2.0,
                                    scalar2=None, op0=Alu.mult)

            # ---------------- X load, Y = dis*X, allgather ----------------
            x_sb = pers.tile([P, RB, P], f32, tag="x")
            nc.sync.dma_start(out=x_sb[:], in_=x_r[:])

            yag_in = dram.tile([R_PAD, P], f32)
            yag_in_r = yag_in[:].rearrange("(b p) f -> p b f", p=P)
            for b in range(RB):
                yt = work.tile([P, P], f32, tag="yt")
                nc.vector.tensor_scalar(out=yt[:], in0=x_sb[:, b, :],
                                        scalar1=dis[:, b:b + 1], scalar2=None,
                                        op0=Alu.mult)
                nc.sync.dma_start(out=yag_in_r[:, b, :], in_=yt[:])
            y_full = dram.tile([NFULL, P], f32, addr_space="Shared")
            nc.gpsimd.collective_compute(
                "AllGather", Alu.bypass,
                replica_groups=[list(range(NCORES))],
                ins=[yag_in.opt()], outs=[y_full.opt()])

            # shared SpMM: gathers from src (DRAM, NFULL x P), one-hot matmul
            # scatter into a per-block PSUM accumulator.
            def spmm_block(b, src_ap, psum_t):
                nmm = int(G[b, 0] + G[b, 1])
                i = 0
                for h in (0, 1):
                    gs = int(G[b, h])
                    soff = int(seg_start[b, h])         # edge offset
                    goff = soff // P                    # group offset
                    v = vpool.tile([P, int(G.max()), P], f32, tag="v")
                    nc.gpsimd.dma_gather(
                        out_ap=v[:, :gs, :],
                        in_ap=src_ap[h * HALF:(h + 1) * HALF, :],
                        idxs_ap=idx_sb[:, soff // 16:(soff + gs * P) // 16],
                        num_idxs=gs * P, num_idxs_reg=gs * P, elem_size=P)
                    mt = mtpool.tile([P, int(G.max()) * P], f32, tag="mt")
                    mtv = mt[:, :gs * P].rearrange("p (g e) -> p g e", e=P)
                    nc.vector.tensor_tensor(
                        out=mtv, in0=iota_f[:].unsqueeze(1).to_broadcast([P, gs, P]),
                        in1=lr_sb[:, goff:goff + gs].to_broadcast([P, gs, P]),
                        op=Alu.is_equal)
                    nc.vector.tensor_tensor(
                        out=mtv, in0=mtv,
                        in1=w_sb[:, goff:goff + gs].to_broadcast([P, gs, P]),
                        op=Alu.mult)
                    for g in range(gs):
                        nc.tensor.matmul(
                            out=psum_t[:], lhsT=mt[:, g * P:(g + 1) * P],
                            rhs=v[:, g, :], start=(i == 0), stop=(i == nmm - 1))
                        i += 1

            idx_sb = pers.tile([P, TG * 8], mybir.dt.int16, tag="idx")
            nc.sync.dma_start(out=idx_sb[:], in_=idx_all[:])
            lr_sb = pers.tile([P, TG], f32, tag="lr")
            nc.sync.dma_start(out=lr_sb[:], in_=lr_all[:])
            w_sb = pers.tile([P, TG], f32, tag="w")
            nc.sync.dma_start(out=w_sb[:], in_=w_all[:])

            # ---------------- SpMM 1: U1 = S(Y) ----------------
            u1_sb = pers.tile([P, RB, P], f32, tag="u1")
            for b in range(RB):
                ps = ppool.tile([P, P], f32, tag="u", space="PSUM")
                spmm_block(b, y_full[:], ps)
                nc.scalar.copy(out=u1_sb[:, b, :], in_=ps[:])

            # ---------------- Y2 = dis^2*U1, allgather; A = -dis*U1 --------
            y2ag_in = dram.tile([R_PAD, P], f32)
            y2ag_in_r = y2ag_in[:].rearrange("(b p) f -> p b f", p=P)
            for b in range(RB):
                yt = work.tile([P, P], f32, tag="yt")
                nc.vector.tensor_scalar(out=yt[:], in0=u1_sb[:, b, :],
                                        scalar1=dis2[:, b:b + 1], scalar2=None,
                                        op0=Alu.mult)
                nc.sync.dma_start(out=y2ag_in_r[:, b, :], in_=yt[:])
            y2_full = dram.tile([NFULL, P], f32, addr_space="Shared")
            nc.gpsimd.collective_compute(
                "AllGather", Alu.bypass,
                replica_groups=[list(range(NCORES))],
                ins=[y2ag_in.opt()], outs=[y2_full.opt()])
            # A = -dis * U1 (in place; only read after this point)
            for b in range(RB):
                nc.vector.tensor_scalar(out=u1_sb[:, b, :], in0=u1_sb[:, b, :],
                                        scalar1=ndis[:, b:b + 1], scalar2=None,
                                        op0=Alu.mult)

            # ---------------- SpMM 2 + gates, fused per block --------------
            for b in range(RB):
                ps2 = ppool.tile([P, P], f32, tag="u", space="PSUM")
                spmm_block(b, y2_full[:], ps2)
                bt_sb = work.tile([P, P], f32, tag="bt")
                nc.vector.tensor_scalar(out=bt_sb[:], in0=ps2[:],
                                        scalar1=dis2x[:, b:b + 1], scalar2=None,
                                        op0=Alu.mult)
                # feature-major transposes of X, A(=Tx1), B
                tmats = []
                for src, tag in ((x_sb[:, b, :], "xt"), (u1_sb[:, b, :], "at"),
                                 (bt_sb[:], "bt2")):
                    tp = tpsum.tile([P, P], f32, tag="tp", space="PSUM")
                    nc.tensor.transpose(out=tp[:], in_=src, identity=ident[:])
                    ts = work.tile([P, P], f32, tag=tag)
                    nc.scalar.copy(out=ts[:], in_=tp[:])
                    tmats.append(ts)
                xt, at, btm = tmats
                gate_ps = {}
                for g in GATES:
                    pg = gpsum.tile([P, P], f32, tag="g", space="PSUM")
                    nc.tensor.matmul(out=pg[:], lhsT=xt[:], rhs=w0m[g][:],
                                     start=True, stop=False)
                    nc.tensor.matmul(out=pg[:], lhsT=at[:], rhs=wsb[(g, 1)][:],
                                     start=False, stop=False)
                    nc.tensor.matmul(out=pg[:], lhsT=btm[:], rhs=wsb[(g, 2)][:],
                                     start=False, stop=False)
                    nc.tensor.matmul(out=pg[:], lhsT=ones1[:],
                                     rhs=bias_sb[:, g * P:(g + 1) * P],
                                     start=False, stop=True)
                    gate_ps[g] = pg
                i_t = work.tile([P, P], f32, tag="i")
                nc.scalar.activation(out=i_t[:], in_=gate_ps[0][:], func=Act.Sigmoid)
                tt_t = work.tile([P, P], f32, tag="tt")
                nc.scalar.activation(out=tt_t[:], in_=gate_ps[2][:], func=Act.Tanh)
                c_t = work.tile([P, P], f32, tag="c")
                nc.vector.tensor_tensor(out=c_t[:], in0=i_t[:], in1=tt_t[:],
                                        op=Alu.mult)
                wcc = work.tile([P, P], f32, tag="wcc")
                nc.vector.tensor_tensor(out=wcc[:], in0=c_t[:], in1=wc2_rep[:],
                                        op=Alu.mult)
                oin = work.tile([P, P], f32, tag="oin")
                nc.vector.tensor_tensor(out=oin[:], in0=gate_ps[3][:], in1=wcc[:],
                                        op=Alu.add)
                o_t = work.tile([P, P], f32, tag="o")
                nc.scalar.activation(out=o_t[:], in_=oin[:], func=Act.Sigmoid)
                tc_t = work.tile([P, P], f32, tag="tc")
                nc.scalar.activation(out=tc_t[:], in_=c_t[:], func=Act.Tanh)
                h_t = work.tile([P, P], f32, tag="h")
                nc.vector.tensor_tensor(out=h_t[:], in0=o_t[:], in1=tc_t[:],
                                        op=Alu.mult)
                res = work.tile([P, P], f32, tag="res")
                nc.scalar.activation(out=res[:], in_=h_t[:], func=Act.Relu)
                nc.sync.dma_start(out=out_r[:, b, :], in_=res[:])

    nc.compile()
    return nc


# ----------------------------------------------------------------------------
# Entry point
# ----------------------------------------------------------------------------

_CACHE = {}


def _get_built(cfg_key, cfg):
    if cfg_key not in _CACHE:
        _CACHE[cfg_key] = _build(cfg)
    return _CACHE[cfg_key]


def _make_in_maps(inputs):
    node_feats = np.asarray(inputs["node_feats"])
    edge_feats = np.asarray(inputs["edge_feats"], np.float32)
    edge_index = np.asarray(inputs["edge_index"])
    t = node_feats.shape[0] - 1
    X = np.asarray(node_feats[t], np.float32)
    row = np.asarray(edge_index[t, 0], np.int64)
    col = np.asarray(edge_index[t, 1], np.int64)
    w = np.asarray(edge_feats[t], np.float32)

    in_maps, cfg = _preprocess(X, row, col, w)

    Wx = np.asarray(inputs["Wx"], np.float32)
    bsum = (np.asarray(inputs["bx"], np.float32)
            + np.asarray(inputs["bh"], np.float32)
            + np.asarray(inputs["bg"], np.float32)).reshape(1, -1)
    wc = np.asarray(inputs["wc"], np.float32).reshape(1, -1)
    for m in in_maps:
        m["wx_t"] = Wx
        m["bsum_t"] = bsum
        m["wc_t"] = wc
    return in_maps, cfg


def _run(inputs, trace=False):
    from concourse.bass_utils import run_bass_kernel_spmd

    in_maps, cfg = _make_in_maps(inputs)
    key = (cfg["N"], cfg["RB"], cfg["DEG_PAD"], cfg["TG"],
           tuple(cfg["G"].ravel().tolist()))
    nc = _get_built(key, cfg)
    res = run_bass_kernel_spmd(nc, in_maps, core_ids=list(range(NCORES)),
                               trace=trace)
    N, R, R_PAD = cfg["N"], cfg["R"], cfg["R_PAD"]
    out = np.empty((N, P), np.float32)
    for c in range(NCORES):
        lo, hi = c * R, min((c + 1) * R, N)
        out[lo:hi] = res.results[c]["out_loc"][: hi - lo]
    return out, res.exec_time_ns


def kernel(**inputs) -> np.ndarray:
    out, _ = _run(inputs, trace=False)
    return out
